# revision 2
# baseline (speedup 1.0000x reference)
"""Trainium2 Bass kernel for the LocalConnectivity diamond-ring stencil.

out[b, x, y] = sum_{1<=|dx|+|dy|<=5} w[|dx|+|dy|-1] * in[b, (x+dx)%512, (y+dy)%512]

Strategy
--------
Data-parallel over batch: 64 samples -> 8 cores x 8 samples. Per sample the
512x512 grid is processed in 5 row-tiles (~103 output rows each). The whole
60-tap stencil runs on the TensorEngine as 11 PSUM-accumulating matmuls, one
per horizontal shift dy in [-5, 5]:

  psum[p, f] += sum_c  WB_dy[c, p] * X[c, f + dy_idx]

where X is the input tile with 5 halo rows on each side (contraction dim =
nrows+10 partitions) and 5 circular halo columns on each side (so horizontal
shifts are free-dim AP offsets), and WB_dy is the banded Toeplitz matrix
holding the vertical taps of kernel column dy: WB_dy[c, p] = K(c-p-5, dy).

float32r keeps the PE at 1 cycle/row while multiplying at FP22 (~1e-4 rel
err). PSUM (fp32) accumulates exactly; ScalarE/VectorE alternate evicting
PSUM -> SBUF; HW-DGE DMA moves HBM <-> SBUF.
"""

import numpy as np

import concourse.bass as bass
import concourse.bacc as bacc
import concourse.mybir as mybir
from concourse import tile
from concourse.bass_utils import run_bass_kernel_spmd

B, H, W = 64, 512, 512
NCORES = 8
BPC = B // NCORES  # samples per core
MAXD = 5
HALO = MAXD
DYS = 2 * MAXD + 1  # 11 horizontal shifts
# (row0, nrows) output row tiles; nrows + 2*HALO <= 128 contraction rows
ROW_TILES = [(0, 103), (103, 103), (206, 103), (309, 103), (412, 100)]


def _build_band_weights(dw: np.ndarray) -> np.ndarray:
    """[128, 11*128] f32: WB[c, j*128 + p] = K(c-p-5, j-5)."""
    wb = np.zeros((128, DYS, 128), dtype=np.float32)
    p = np.arange(128)
    for j in range(DYS):
        dy = j - MAXD
        for dx in range(-MAXD, MAXD + 1):
            d = abs(dx) + abs(dy)
            if 1 <= d <= MAXD:
                c = p + dx + HALO
                valid = (c >= 0) & (c < 128)
                wb[c[valid], j, p[valid]] = dw[d - 1]
    return np.ascontiguousarray(wb.reshape(128, DYS * 128))


_CACHED_NC = None


def _build_program():
    f32 = mybir.dt.float32
    f32r = mybir.dt.float32r

    nc = bacc.Bacc(None, target_bir_lowering=False)
    x = nc.dram_tensor("x", [BPC, H, W], f32r, kind="ExternalInput")
    wb = nc.dram_tensor("wb", [128, DYS * 128], f32r, kind="ExternalInput")
    y = nc.dram_tensor("y", [BPC, H, W], f32, kind="ExternalOutput")

    with tile.TileContext(nc) as tc:
        with (
            tc.tile_pool(name="wpool", bufs=1) as wpool,
            tc.tile_pool(name="xpool", bufs=4) as xpool,
            tc.tile_pool(name="opool", bufs=4) as opool,
            tc.tile_pool(name="pspool", bufs=6, space=bass.MemorySpace.PSUM) as pspool,
        ):
            wtile = wpool.tile([128, DYS * 128], f32r)
            nc.sync.dma_start(wtile[:], wb[:])

            k = 0
            for b in range(BPC):
                for r0, nrows in ROW_TILES:
                    ctr = nrows + 2 * HALO  # contraction rows incl. halo
                    xt = xpool.tile([128, W + 2 * HALO], f32r)

                    # circular row range [r0-HALO, r0-HALO+ctr) mod H,
                    # split into 1-2 contiguous runs
                    start = (r0 - HALO) % H
                    if start + ctr <= H:
                        runs = [(0, start, ctr)]
                    else:
                        runs = [
                            (0, start, H - start),
                            (H - start, 0, ctr - (H - start)),
                        ]
                    for off, gs, glen in runs:
                        nc.sync.dma_start(
                            xt[off : off + glen, HALO : HALO + W],
                            x[b, gs : gs + glen, :],
                        )
                        nc.sync.dma_start(
                            xt[off : off + glen, 0:HALO],
                            x[b, gs : gs + glen, W - HALO : W],
                        )
                        nc.sync.dma_start(
                            xt[off : off + glen, HALO + W :],
                            x[b, gs : gs + glen, 0:HALO],
                        )

                    pt = pspool.tile([128, W], f32)
                    for j in range(DYS):
                        nc.tensor.matmul(
                            pt[0:nrows, :],
                            wtile[0:ctr, j * 128 : j * 128 + nrows],
                            xt[0:ctr, j : j + W],
                            start=(j == 0),
                            stop=(j == DYS - 1),
                        )

                    ot = opool.tile([128, W], f32)
                    if k % 2 == 0:
                        nc.scalar.copy(ot[0:nrows, :], pt[0:nrows, :])
                    else:
                        nc.vector.tensor_copy(ot[0:nrows, :], pt[0:nrows, :])
                    nc.sync.dma_start(y[b, r0 : r0 + nrows, :], ot[0:nrows, :])
                    k += 1
    nc.compile()
    return nc


def _get_program():
    global _CACHED_NC
    if _CACHED_NC is None:
        _CACHED_NC = _build_program()
    return _CACHED_NC


def _run(grid_spikes, distance_weights, trace=False):
    grid_spikes = np.ascontiguousarray(np.asarray(grid_spikes, dtype=np.float32))
    distance_weights = np.asarray(distance_weights, dtype=np.float32)
    assert grid_spikes.shape == (B, H, W), grid_spikes.shape
    wb_np = _build_band_weights(distance_weights)

    nc = _get_program()
    in_maps = [
        {
            "x": np.ascontiguousarray(grid_spikes[i * BPC : (i + 1) * BPC]),
            "wb": wb_np,
        }
        for i in range(NCORES)
    ]
    res = run_bass_kernel_spmd(nc, in_maps, list(range(NCORES)), trace=trace)
    out = np.concatenate([res.results[i]["y"] for i in range(NCORES)], axis=0)
    return out.astype(np.float32, copy=False), res


def kernel(grid_spikes, distance_weights):
    out, _ = _run(grid_spikes, distance_weights, trace=False)
    return out


def kernel_traced(grid_spikes, distance_weights):
    out, res = _run(grid_spikes, distance_weights, trace=True)
    return out, res


# revision 5
# speedup vs baseline: 1.8780x; 1.8780x over previous
"""Trainium2 Bass kernel for the LocalConnectivity diamond-ring stencil.

out[b, x, y] = sum_{1<=|dx|+|dy|<=5} w[|dx|+|dy|-1] * in[b, (x+dx)%512, (y+dy)%512]

Strategy
--------
Data-parallel over batch: 64 samples -> 8 cores x 8 samples. Per sample the
512x512 grid is processed in 5 row-tiles (~103 output rows each). The whole
60-tap stencil runs on the TensorEngine as 11 PSUM-accumulating matmuls, one
per horizontal shift dy in [-5, 5]:

  psum[p, f] += sum_c  WB_dy[c, p] * X[c, f + dy_idx]

where X is the input tile with 5 halo rows on each side (contraction dim =
nrows+10 partitions) and 5 circular halo columns on each side (so horizontal
shifts are free-dim AP offsets), and WB_dy is the banded Toeplitz matrix
holding the vertical taps of kernel column dy: WB_dy[c, p] = K(c-p-5, dy).

float32r keeps the PE at 1 cycle/row while multiplying at FP22 (~1e-4 rel
err). PSUM (fp32) accumulates exactly; ScalarE/VectorE alternate evicting
PSUM -> SBUF; HW-DGE DMA moves HBM <-> SBUF.
"""

import numpy as np

import concourse.bass as bass
import concourse.bacc as bacc
import concourse.mybir as mybir
from concourse import tile
from concourse.bass_utils import run_bass_kernel_spmd

B, H, W = 64, 512, 512
NCORES = 8
BPC = B // NCORES  # samples per core
MAXD = 5
HALO = MAXD
DYS = 2 * MAXD + 1  # 11 horizontal shifts
# (row0, nrows) output row tiles; nrows + 2*HALO <= 128 contraction rows
ROW_TILES = [(0, 103), (103, 103), (206, 103), (309, 103), (412, 100)]


def _build_band_weights(dw: np.ndarray) -> np.ndarray:
    """[128, 11*128] f32: WB[c, j*128 + p] = K(c-p-5, j-5)."""
    wb = np.zeros((128, DYS, 128), dtype=np.float32)
    p = np.arange(128)
    for j in range(DYS):
        dy = j - MAXD
        for dx in range(-MAXD, MAXD + 1):
            d = abs(dx) + abs(dy)
            if 1 <= d <= MAXD:
                c = p + dx + HALO
                valid = (c >= 0) & (c < 128)
                wb[c[valid], j, p[valid]] = dw[d - 1]
    return np.ascontiguousarray(wb.reshape(128, DYS * 128))


_CACHED_NC = None


def _build_program():
    f32 = mybir.dt.float32
    f32r = mybir.dt.float32r

    nc = bacc.Bacc(None, target_bir_lowering=False)
    x = nc.dram_tensor("x", [BPC, H, W], f32r, kind="ExternalInput")
    wb = nc.dram_tensor("wb", [128, DYS * 128], f32r, kind="ExternalInput")
    y = nc.dram_tensor("y", [BPC, H, W], f32, kind="ExternalOutput")

    with tile.TileContext(nc) as tc:
        with (
            tc.tile_pool(name="wpool", bufs=1) as wpool,
            tc.tile_pool(name="xpool", bufs=6) as xpool,
            tc.tile_pool(name="opool", bufs=4) as opool,
            tc.tile_pool(name="pspool", bufs=8, space=bass.MemorySpace.PSUM) as pspool,
        ):
            wtile = wpool.tile([128, DYS * 128], f32r)
            nc.sync.dma_start(wtile[:], wb[:])

            k = 0
            for b in range(BPC):
                for r0, nrows in ROW_TILES:
                    ctr = nrows + 2 * HALO  # contraction rows incl. halo
                    xt = xpool.tile([128, W + 2 * HALO], f32r)

                    # circular row range [r0-HALO, r0-HALO+ctr) mod H,
                    # split into 1-2 contiguous runs
                    start = (r0 - HALO) % H
                    if start + ctr <= H:
                        runs = [(0, start, ctr)]
                    else:
                        runs = [
                            (0, start, H - start),
                            (H - start, 0, ctr - (H - start)),
                        ]
                    for off, gs, glen in runs:
                        nc.sync.dma_start(
                            xt[off : off + glen, HALO : HALO + W],
                            x[b, gs : gs + glen, :],
                        )
                    # circular column halos via on-chip copies (tiny DMAs
                    # are ~10x slower): cols 507..511 -> g 0..4, cols 0..4
                    # -> g 517..521
                    nc.vector.tensor_copy(
                        xt[0:ctr, 0:HALO], xt[0:ctr, W : W + HALO]
                    )
                    nc.vector.tensor_copy(
                        xt[0:ctr, HALO + W :], xt[0:ctr, HALO : 2 * HALO]
                    )

                    pt = pspool.tile([128, W], f32)
                    for j in range(DYS):
                        nc.tensor.matmul(
                            pt[0:nrows, :],
                            wtile[0:ctr, j * 128 : j * 128 + nrows],
                            xt[0:ctr, j : j + W],
                            start=(j == 0),
                            stop=(j == DYS - 1),
                        )

                    ot = opool.tile([128, W], f32)
                    nc.scalar.copy(ot[0:nrows, :], pt[0:nrows, :])
                    nc.gpsimd.dma_start(y[b, r0 : r0 + nrows, :], ot[0:nrows, :])
                    k += 1
    nc.compile()
    return nc


def _get_program():
    global _CACHED_NC
    if _CACHED_NC is None:
        _CACHED_NC = _build_program()
    return _CACHED_NC


def _run(grid_spikes, distance_weights, trace=False):
    grid_spikes = np.ascontiguousarray(np.asarray(grid_spikes, dtype=np.float32))
    distance_weights = np.asarray(distance_weights, dtype=np.float32)
    assert grid_spikes.shape == (B, H, W), grid_spikes.shape
    wb_np = _build_band_weights(distance_weights)

    nc = _get_program()
    in_maps = [
        {
            "x": np.ascontiguousarray(grid_spikes[i * BPC : (i + 1) * BPC]),
            "wb": wb_np,
        }
        for i in range(NCORES)
    ]
    res = run_bass_kernel_spmd(nc, in_maps, list(range(NCORES)), trace=trace)
    out = np.concatenate([res.results[i]["y"] for i in range(NCORES)], axis=0)
    return out.astype(np.float32, copy=False), res


def kernel(grid_spikes, distance_weights):
    out, _ = _run(grid_spikes, distance_weights, trace=False)
    return out


def kernel_traced(grid_spikes, distance_weights):
    out, res = _run(grid_spikes, distance_weights, trace=True)
    return out, res
